# revision 42
# baseline (speedup 1.0000x reference)
"""Trainium2 Bass kernel for nn_CATransformer1 (XCiT-style channel-attention block).

Sharding: data-parallel over batch. 16 images / 8 cores = 2 images per core.
Weights are replicated; no collectives. Each core computes its 2 images fully.

v5 (0.855 ms) vs baseline (1.30 ms); rel err 3.3e-3 (budget 2e-2):
  - Single f32r DMA load per x chunk (DVE reads f32r directly); bf16 for
    FFN/G-build/rank-1 matmuls, y/h/yn/qksq/mneg tiles; f32 PSUM accumulate.
  - 512-pixel chunks; qk eviction on scalar (Copy+scale; no table churn:
    8 ACT_TABLE_LOADs vs 68).  Pass B split into loop1 (attn+residual+stats,
    y resident in SBUF) and loop2 (yn+FFN, GELU-only scalar).
  - rstd/m2/rstd2 rows round-trip DRAM; loop1/2 replicate them across
    partitions with stride-0 bcast_read DMAs (no PE broadcast matmuls).
  - Per-pixel 1/std: PE-transpose std row segments + one 128-lane reciprocal.
  - Batched softmax (3D tensor_reduce / broadcast ops, one Exp) and batched
    sS scaling; row math fused into Square(scale=1/sqrt(C)) + sub +
    Sqrt(scale=1/C, bias=eps).
  - qk tile bf16: the 1024 S/norm matmuls get FWL weight loads and the
    qksq square runs at 2x DVE rate (951us -> 885us, the single biggest
    win of the tuning passes).
  - qk-projection matmuls bf16 too (wqk host-cast; x chunk cast on-chip by
    one DVE copy; LN1 stats keep the f32r x): 885us -> 855us.
  Remaining (perfetto, at 951us): ~65us of >1us PE-idle gaps (loop1
  inter-chunk, attn-build hole) keep N=512 matmuls ~295 ns avg vs 216
  warm; single-lane row reciprocals 92us fill vector idle.
  Tried and REGRESSED (do not retry as-is): column->row PE-transpose +
  fast-DMA replacement of the row reciprocals (+40us: PSUM rotation
  pressure, loop1 7 allocs/chunk > 6 bufs), transposing-AP DMA of the
  [128,4] column (128 tiny descriptors, +80us), xg bufs=3 (+20us);
  xc bufs=3 overflows SBUF by 0.25KB (pool carries xc+xsq tags, 8KB/buf
  vs 7.75KB slack).
  Tried and BLOCKED: S-matmul head pairing (odd head lands at partition
  base 48 - illegal for ACT/DVE access; needs a 16-col padded qk layout
  so pairs align to base 64); odd-head S to base 64 via
  tile_position=(0,64) compiles and runs but MIS-ACCUMULATES (rel err
  3.3e-3 -> 1.7e-2) and is slower (+130us) - the f32 matmul path does
  not compose with col-group tiling here; custom-ISA ops (walrus "ISA wrong length":
  no reciprocal_approx_fast / partition_broadcast), f32->f32r bitcast
  (BIR verifier requires a rounding producer), DVE divide ALU (no
  codegen), stride-0-partition SBUF DMA broadcast (AP assert).  SBUF is
  ~2KB from full; PSUM is 8/8 banks (ps 6 + psacc 2).
"""

import numpy as np

B, C, NH, CH, N, HID = 16, 384, 8, 48, 4096, 1536
NCORES = 8
BPC = B // NCORES  # images per core
P = 128
KS = C // P  # 3 k-subtiles for C
KH = HID // P  # 12 k-subtiles for HID
LOGIT_MAX = float(np.log(1.0 / 0.01))
EPS_LN = 1e-5
EPS_NORM = 1e-12

_CACHE = {}


def _patch_tile_drain():
    """Walrus in this env rejects >1 sync-wait on the kernel-tail Drain
    (CTRL_NO_STRUCT setupSyncWait).  Split the waits across a chain of
    drain instructions, one wait each.  Idempotent, in-process only."""
    import concourse.tile as tile
    from concourse import mybir
    from concourse.vector_clock import ScopedClock

    if getattr(tile.TileContext._drain_and_barrier, "_split_patch", False):
        return

    def _split_drain(self, tick_clock, wait_clock):
        drain_inst = self.nc.sync.drain()
        wait_clock.add_sem_waits(
            drain_inst.ins, ScopedClock({None: tick_clock.global_clock}))
        si = drain_inst.ins.sync_info
        if si is not None and si.on_wait and len(si.on_wait) > 1:
            waits = list(si.on_wait)
            si.on_wait = waits[:1]
            for w in waits[1:]:
                d2 = self.nc.sync.drain()
                d2.ins.sync_info = mybir.SyncInfo(on_wait=[w], on_update=[])
        self.nc.all_engine_barrier()
        popped = self.nc._tile_sem_poison_stack.pop()
        assert popped is self._sem_poison
        self.nc.clear_and_free_semaphores(list(self.sems.allocated().values()))
        self.nc.all_engine_barrier()

    _split_drain._split_patch = True
    tile.TileContext._drain_and_barrier = _split_drain


def _split_waits(nc, max_waits=1):
    """This walrus build rejects instructions carrying more than one sync
    wait ('Too many sync wait commands' / 'ISA wrong length').  Move extra
    waits onto same-engine NoOps inserted immediately before."""
    from concourse import mybir

    n = 0
    for fn in nc.m.functions:
        for blk in fn.blocks:
            out = []
            for inst in blk.instructions:
                si = inst.sync_info
                lim = 0 if type(inst).__name__ == "InstISA" else max_waits
                if si is not None and si.on_wait and len(si.on_wait) > lim:
                    waits = list(si.on_wait)
                    keep = waits[-lim:] if lim else []
                    for w in waits[: len(waits) - lim]:
                        n += 1
                        nop = mybir.InstNoOp(
                            name=f"I-wsplit-{n}", ins=[], outs=[])
                        nop.engine = inst.engine
                        nop.sync_info = mybir.SyncInfo(
                            on_wait=[w], on_update=[])
                        out.append(nop)
                    si.on_wait = keep
                out.append(inst)
            blk.instructions = out
    return nc


def _build_nc():
    import concourse.bass as bass
    import concourse.tile as tile
    from concourse import mybir

    dt = mybir.dt
    AF = mybir.ActivationFunctionType
    ALU = mybir.AluOpType
    AX = mybir.AxisListType
    from concourse.masks import make_identity

    f32 = dt.float32
    f32r = dt.float32r
    bf16 = dt.bfloat16

    def R(ap):
        return ap.bitcast(f32r)

    _patch_tile_drain()
    nc = bass.Bass()

    xs = nc.declare_dram_parameter("xs", [BPC, C, N], f32, isOutput=False)
    wqk_t = nc.declare_dram_parameter("wqk_t", [C, 2 * C], bf16, isOutput=False)
    u_qk = nc.declare_dram_parameter("u_qk", [1, 2 * C], bf16, isOutput=False)
    wv = nc.declare_dram_parameter("wv", [CH, NH, C], bf16, isOutput=False)
    wpj48 = nc.declare_dram_parameter("wpj48", [CH, NH, C], bf16, isOutput=False)
    w1_t = nc.declare_dram_parameter("w1_t", [C, HID], bf16, isOutput=False)
    w2_t = nc.declare_dram_parameter("w2_t", [HID, C], bf16, isOutput=False)
    scale_row = nc.declare_dram_parameter("scale_row", [1, NH], f32, isOutput=False)
    out_d = nc.declare_dram_parameter("out", [BPC, C, N], f32, isOutput=True)

    FC = 512   # pass-A pixel chunk
    NFC = N // FC          # 8
    TPC = FC // P          # 4   128-px tiles per chunk
    FG = 512   # pass-B pixel chunk
    NFG = N // FG          # 8
    NT = N // P            # 32  128-px tiles per image

    with tile.TileContext(nc) as tc:
        with (
            tc.tile_pool(name="consts", bufs=1) as consts,
            tc.tile_pool(name="xc", bufs=2) as xcp,
            tc.tile_pool(name="xg", bufs=2) as xgp,
            tc.tile_pool(name="qk", bufs=2) as qkpool,
            tc.tile_pool(name="attn", bufs=1) as apool,
            tc.tile_pool(name="gt", bufs=1) as gtp,
            tc.tile_pool(name="workA", bufs=2) as work,
            tc.tile_pool(name="yimg", bufs=1) as yip,
            tc.tile_pool(name="hb", bufs=1) as hbp,
            tc.tile_pool(name="yout", bufs=2) as youtp,
            tc.tile_pool(name="small", bufs=1) as small,
            tc.tile_pool(name="rows", bufs=1) as rowp,
            tc.tile_pool(name="ps", bufs=6, space="PSUM") as ps,
            tc.tile_pool(name="dram", bufs=2, space="DRAM") as dramp,
            tc.tile_pool(name="psacc", bufs=1, space="PSUM") as psacc,
        ):
            def bcast_read(dst, dram_row, parts=P):
                src = bass.AP(
                    tensor=dram_row.tensor, offset=dram_row.offset,
                    ap=[[0, parts]] + [list(d) for d in dram_row.ap[-1:]])
                nc.gpsimd.dma_start(dst, src)

            # ---------------- constants ----------------
            wqk_sb = consts.tile([P, KS, 2 * C], bf16, tag="wqk")
            nc.sync.dma_start(wqk_sb[:], wqk_t.rearrange("(s p) f -> p s f", p=P))
            wv_b = consts.tile([CH, NH, C], bf16, tag="wv")
            nc.sync.dma_start(wv_b[:], wv[:])
            wpj_b = consts.tile([CH, NH, C], bf16, tag="wpj")
            nc.sync.dma_start(wpj_b[:], wpj48[:])
            w1_b = consts.tile([P, KS, HID], bf16, tag="w1")
            nc.sync.dma_start(w1_b[:], w1_t.rearrange("(s p) f -> p s f", p=P))
            w2_b = consts.tile([P, KH, C], bf16, tag="w2")
            nc.sync.dma_start(w2_b[:], w2_t.rearrange("(s p) f -> p s f", p=P))
            uqk_b = consts.tile([1, 2 * C], bf16, tag="uqk")
            nc.sync.dma_start(uqk_b[:], u_qk[:])
            ones_c = consts.tile([P, KS, 1], f32, tag="ones")
            nc.vector.memset(ones_c[:], 1.0)
            ones_r = consts.tile([P, KS, 1], f32r, tag="onesr")
            nc.vector.tensor_copy(ones_r[:], ones_c[:])
            ones_b = consts.tile([P, KS, 1], bf16, tag="onesb")
            nc.vector.tensor_copy(ones_b[:], ones_c[:])
            ones2_c = consts.tile([P, 2], f32, tag="ones2")
            nc.vector.memset(ones2_c[:], 1.0)
            ones2_b = consts.tile([P, 2], bf16, tag="ones2b")
            nc.vector.tensor_copy(ones2_b[:], ones2_c[:])
            onesrow_c = consts.tile([1, P], f32, tag="onesrow")
            nc.vector.memset(onesrow_c[:], 1.0)
            onesrow_b = consts.tile([1, P], bf16, tag="onesrowb")
            nc.vector.tensor_copy(onesrow_b[:], onesrow_c[:])
            epsb = consts.tile([1, 1], f32, tag="epsb")
            nc.vector.memset(epsb[:], EPS_LN)
            ident = consts.tile([CH, CH], f32, tag="ident")
            make_identity(nc, ident[:])
            schb = consts.tile([CH, NH], f32, tag="schb")
            bcast_read(schb[:], scale_row[0, :], parts=CH)

            xs_r = xs.rearrange("b (s p) n -> b p s n", p=P)
            out_r = out_d.rearrange("b (s p) n -> b p s n", p=P)

            for img in range(BPC):
                # mneg row in SBUF (rank-1 matmul operand); the broadcast
                # rows (rstd, m2, rstd2) round-trip through DRAM so loop1/2
                # can replicate them across partitions with stride-0 DMA
                # reads instead of PE matmul broadcasts.
                mneg_b = rowp.tile([1, N], bf16, tag="mneg")
                rstd_dram = dramp.tile([1, N], f32, tag="rstdd")
                m2_dram = dramp.tile([1, N], f32, tag="m2d")
                r2_dram = dramp.tile([1, N], f32, tag="r2d")

                # ---- pass A: LN1 stats + qkT + S/norm accumulation ----
                ps_s = psacc.tile([CH, NH * CH + 2 * NH], f32, tag="psS")
                ps_nk = psacc.tile([1, C], f32, tag="psnk")
                for f in range(NFC):
                    sl = slice(f * FC, (f + 1) * FC)
                    xc = xcp.tile([P, KS, FC], f32r, tag="xc")
                    nc.gpsimd.dma_start(xc[:], xs_r[img][:, :, sl])
                    xcb = xcp.tile([P, KS, FC], bf16, tag="xcb")
                    nc.vector.tensor_copy(xcb[:], xc[:])
                    prow = ps.tile([1, FC], f32, tag="pb")
                    prow2 = ps.tile([1, FC], f32, tag="pb")
                    for s in range(KS):
                        nc.tensor.matmul(
                            prow[0:1, :], ones_r[:, s, :], xc[:, s, :],
                            start=(s == 0), stop=(s == KS - 1))
                    for s in range(KS):
                        xsq = xcp.tile([P, FC], f32r, tag="xsq")
                        nc.vector.tensor_mul(xsq[:], xc[:, s, :], xc[:, s, :])
                        nc.tensor.matmul(
                            prow2[0:1, :], ones_r[:, s, :], xsq[:],
                            start=(s == 0), stop=(s == KS - 1))
                    # row math
                    nc.vector.tensor_scalar(
                        mneg_b[0:1, sl], prow[0:1, :], -1.0 / C, None,
                        op0=ALU.mult)
                    vrow = small.tile([1, FC], f32, tag="vrow")
                    nc.vector.tensor_scalar(
                        vrow[:], prow2[0:1, :], 1.0 / C, EPS_LN,
                        op0=ALU.mult, op1=ALU.add)
                    msq = small.tile([1, FC], f32, tag="msq")
                    nc.scalar.activation(
                        msq[:], prow[0:1, :], AF.Square, scale=-1.0 / C)
                    nc.vector.tensor_sub(vrow[:], vrow[:], msq[:])
                    srow = small.tile([1, FC], f32, tag="srow")
                    nc.scalar.activation(srow[:], vrow[:], AF.Sqrt)
                    # transpose the std row -> [128, TPC] column, then a
                    # 128-lane reciprocal (fast); the row-form reciprocal is
                    # only consumed by loop1 (off the pass-A critical path).
                    rcol_ps = ps.tile([P, TPC], f32, tag="pb")
                    for t in range(TPC):
                        nc.tensor.transpose(
                            rcol_ps[:, t : t + 1], srow[0:1, t * P : (t + 1) * P],
                            ident[0:1, 0:1])
                    rcol = small.tile([P, TPC], f32, tag="rcol")
                    nc.vector.reciprocal(rcol[:], rcol_ps[:])
                    rr = small.tile([1, FC], f32, tag="rr")
                    nc.vector.reciprocal(rr[:], srow[:])
                    nc.sync.dma_start(rstd_dram[0:1, sl], rr[:])

                    for t in range(TPC):
                        tt = f * TPC + t
                        tsl = slice(t * P, (t + 1) * P)
                        gsl = slice(f * FC + t * P, f * FC + (t + 1) * P)
                        pa = ps.tile([P, 512], f32, tag="pb")
                        pb = ps.tile([P, 256], f32, tag="pb")
                        for s in range(KS):
                            nc.tensor.matmul(
                                pa[:], xcb[:, s, tsl], wqk_sb[:, s, 0:512],
                                start=(s == 0), stop=False)
                        nc.tensor.matmul(
                            pa[:], mneg_b[0:1, gsl], uqk_b[:, 0:512],
                            start=False, stop=True)
                        for s in range(KS):
                            nc.tensor.matmul(
                                pb[:], xcb[:, s, tsl], wqk_sb[:, s, 512:768],
                                start=(s == 0), stop=False)
                        nc.tensor.matmul(
                            pb[:], mneg_b[0:1, gsl], uqk_b[:, 512:768],
                            start=False, stop=True)
                        qk = qkpool.tile([P, 2 * C], bf16, tag="qk")
                        qksq = qkpool.tile([P, 2 * C], bf16, tag="qksq")
                        rc = rcol[:, t : t + 1]
                        nc.scalar.activation(
                            qk[:, 0:512], pa[:], AF.Copy, scale=rc)
                        nc.scalar.activation(
                            qk[:, 512:768], pb[:], AF.Copy, scale=rc)
                        nc.vector.tensor_mul(qksq[:], qk[:], qk[:])
                        st, sp = (tt == 0), (tt == NT - 1)
                        for h in range(NH):
                            o = h * 2 * CH
                            nc.tensor.matmul(
                                ps_s[:, h * CH : (h + 1) * CH],
                                qk[:, o : o + CH],
                                qk[:, o + CH : o + 2 * CH],
                                start=st, stop=sp)
                            nc.tensor.matmul(
                                ps_s[:, C + 2 * h : C + 2 * h + 2],
                                qksq[:, o : o + CH], ones2_b[:, :],
                                start=st, stop=sp)
                        ksq = qksq.rearrange(
                            "p (h two c) -> p h two c", two=2, c=CH)
                        nc.tensor.matmul(
                            ps_nk[:], ones_b[:, 0, :], ksq[:, :, 1, :],
                            start=st, stop=sp)

                # ---------------- attn softmax + G build ----------------
                rq = apool.tile([CH, NH], f32, tag="rq")
                nc.scalar.activation(
                    rq[:],
                    ps_s[:, C : C + 2 * NH]
                    .rearrange("p (h two) -> p h two", two=2)[:, :, 0],
                    AF.Sqrt)
                nc.vector.tensor_scalar_max(rq[:], rq[:], EPS_NORM)
                rqr = apool.tile([CH, NH], f32, tag="rqr")
                nc.vector.reciprocal(rqr[:], rq[:])
                nc.vector.tensor_mul(rqr[:], rqr[:], schb[:])
                rk = apool.tile([1, C], f32, tag="rk")
                nc.scalar.activation(rk[:], ps_nk[:], AF.Sqrt)
                nc.vector.tensor_scalar_max(rk[:], rk[:], EPS_NORM)
                rkr = apool.tile([1, C], f32, tag="rkr")
                nc.vector.reciprocal(rkr[:], rk[:])
                rkr_b = apool.tile([1, C], bf16, tag="rkrb")
                nc.vector.tensor_copy(rkr_b[:], rkr[:])
                rkb_ps = ps.tile([CH, C], f32, tag="pb")
                nc.tensor.matmul(
                    rkb_ps[:], onesrow_b[0:1, :CH], rkr_b[0:1, :],
                    start=True, stop=True)
                sS = apool.tile([CH, C], f32, tag="sS")
                nc.vector.tensor_mul(
                    sS.rearrange("p (h c) -> p h c", c=CH),
                    ps_s[:CH, 0:C].rearrange("p (h c) -> p h c", c=CH),
                    rqr[:, :, None].to_broadcast((CH, NH, CH)))
                nc.vector.tensor_mul(sS[:], sS[:], rkb_ps[:])
                mx = apool.tile([CH, NH], f32, tag="mx")
                esum = apool.tile([CH, NH], f32, tag="esum")
                sSh = sS.rearrange("p (h c) -> p h c", c=CH)
                nc.vector.tensor_reduce(mx[:], sSh, AX.X, ALU.max)
                nc.vector.tensor_sub(
                    sSh, sSh, mx[:, :, None].to_broadcast((CH, NH, CH)))
                nc.scalar.activation(sS[:], sS[:], AF.Exp)
                nc.vector.tensor_reduce(esum[:], sSh, AX.X, ALU.add)
                esr = apool.tile([CH, NH], f32, tag="esr")
                nc.vector.reciprocal(esr[:], esum[:])
                nc.vector.tensor_mul(
                    sSh, sSh, esr[:, :, None].to_broadcast((CH, NH, CH)))
                atT = apool.tile([CH, C], bf16, tag="atT")
                for h in range(NH):
                    hs = slice(h * CH, (h + 1) * CH)
                    ptr = ps.tile([CH, CH], f32, tag="pb")
                    nc.tensor.transpose(ptr[:], sS[:, hs], ident[:])
                    nc.vector.tensor_copy(atT[:, hs], ptr[:])
                awv_b = apool.tile([CH, NH, C], bf16, tag="awv")
                for h in range(NH):
                    paw = ps.tile([CH, C], f32, tag="pb")
                    nc.tensor.matmul(
                        paw[:], atT[:, h * CH : (h + 1) * CH],
                        wv_b[:, h, :], start=True, stop=True)
                    nc.vector.tensor_copy(awv_b[:, h, :], paw[:])
                gt_sb = gtp.tile([P, KS, C], bf16, tag="gt")
                for j in range(KS):
                    pgt = ps.tile([P, C], f32, tag="pb")
                    for h in range(NH):
                        nc.tensor.matmul(
                            pgt[:], awv_b[:, h, j * P : (j + 1) * P],
                            wpj_b[:, h, :], start=(h == 0), stop=(h == NH - 1))
                    nc.vector.tensor_copy(gt_sb[:, j, :], pgt[:])
                ug = gtp.tile([1, C], f32, tag="ug")
                ug_b = gtp.tile([1, C], bf16, tag="ugb")
                pug = ps.tile([1, C], f32, tag="pb")
                for s in range(KS):
                    nc.tensor.matmul(
                        pug[:], ones_b[:, s, :], gt_sb[:, s, :],
                        start=(s == 0), stop=(s == KS - 1))
                nc.vector.tensor_copy(ug[:], pug[:])
                nc.vector.tensor_copy(ug_b[:], pug[:])

                # ---- pass B loop1: attn branch + residual + LN2 stats ----
                y_img = yip.tile([P, KS, N], bf16, tag="y")
                for f in range(NFG):
                    sl = slice(f * FG, (f + 1) * FG)
                    xg = xgp.tile([P, KS, FG], f32r, tag="xg")
                    nc.gpsimd.dma_start(xg[:], xs_r[img][:, :, sl])
                    xgb = xgp.tile([P, KS, FG], bf16, tag="xgb")
                    nc.vector.tensor_copy(xgb[:], xg[:])
                    rb_sb = work.tile([P, FG], f32, tag="rb")
                    bcast_read(rb_sb[:], rstd_dram[0, sl])
                    for j in range(KS):
                        pg = ps.tile([P, FG], f32, tag="pb")
                        for s in range(KS):
                            nc.tensor.matmul(
                                pg[:], gt_sb[:, s, j * P : (j + 1) * P],
                                xgb[:, s, :], start=(s == 0), stop=False)
                        nc.tensor.matmul(
                            pg[:], ug_b[:, j * P : (j + 1) * P],
                            mneg_b[0:1, sl], start=False, stop=True)
                        ab = work.tile([P, FG], f32, tag="ab")
                        nc.vector.tensor_mul(ab[:], pg[:], rb_sb[:])
                        nc.vector.tensor_add(
                            y_img[:, j, sl], xg[:, j, :], ab[:])
                    p2 = ps.tile([1, FG], f32, tag="pb")
                    p2q = ps.tile([1, FG], f32, tag="pb")
                    for s in range(KS):
                        nc.tensor.matmul(
                            p2[0:1, :], ones_b[:, s, :], y_img[:, s, sl],
                            start=(s == 0), stop=(s == KS - 1))
                    for s in range(KS):
                        ysq = work.tile([P, FG], bf16, tag="ysq")
                        nc.scalar.activation(
                            ysq[:], y_img[:, s, sl], AF.Square)
                        nc.tensor.matmul(
                            p2q[0:1, :], ones_b[:, s, :], ysq[:],
                            start=(s == 0), stop=(s == KS - 1))
                    m2row = small.tile([1, FG], f32, tag="m2row")
                    nc.vector.tensor_scalar(
                        m2row[:], p2[0:1, :], -1.0 / C, None,
                        op0=ALU.mult)
                    nc.sync.dma_start(m2_dram[0:1, sl], m2row[:])
                    v2 = small.tile([1, FG], f32, tag="v2")
                    nc.vector.tensor_scalar(
                        v2[:], p2q[0:1, :], 1.0 / C, EPS_LN,
                        op0=ALU.mult, op1=ALU.add)
                    msq2 = small.tile([1, FG], f32, tag="msq2")
                    nc.scalar.activation(
                        msq2[:], p2[0:1, :], AF.Square, scale=-1.0 / C)
                    nc.vector.tensor_sub(v2[:], v2[:], msq2[:])
                    srow2 = small.tile([1, FG], f32, tag="srow2")
                    nc.scalar.activation(srow2[:], v2[:], AF.Sqrt)
                    rr2 = small.tile([1, FG], f32, tag="rr2")
                    nc.vector.reciprocal(rr2[:], srow2[:])
                    nc.sync.dma_start(r2_dram[0:1, sl], rr2[:])

                # ---- pass B loop2: LN2 apply + FFN (GELU-only scalar) ----
                for f in range(NFG):
                    sl = slice(f * FG, (f + 1) * FG)
                    m2bb = work.tile([P, FG], f32, tag="m2bb")
                    bcast_read(m2bb[:], m2_dram[0, sl])
                    r2bb = work.tile([P, FG], f32, tag="r2bb")
                    bcast_read(r2bb[:], r2_dram[0, sl])
                    yn = work.tile([P, KS, FG], bf16, tag="yn")
                    nc.vector.tensor_add(
                        yn[:], y_img[:, :, sl],
                        m2bb[:, None, :].to_broadcast((P, KS, FG)))
                    nc.vector.tensor_mul(
                        yn[:], yn[:],
                        r2bb[:, None, :].to_broadcast((P, KS, FG)))
                    h_sb = hbp.tile([P, KH, FG], bf16, tag="h")
                    for mh in range(KH):
                        ph = ps.tile([P, FG], f32, tag="pb")
                        for s in range(KS):
                            nc.tensor.matmul(
                                ph[:], w1_b[:, s, mh * P : (mh + 1) * P],
                                yn[:, s, :], start=(s == 0), stop=(s == KS - 1))
                        nc.scalar.activation(h_sb[:, mh, :], ph[:], AF.Gelu)
                    yout = youtp.tile([P, KS, FG], f32, tag="yo")
                    for mo in range(KS):
                        po = ps.tile([P, FG], f32, tag="pb")
                        for s in range(KH):
                            nc.tensor.matmul(
                                po[:], w2_b[:, s, mo * P : (mo + 1) * P],
                                h_sb[:, s, :],
                                start=(s == 0), stop=(s == KH - 1))
                        nc.vector.tensor_add(
                            yout[:, mo, :], po[:], y_img[:, mo, sl])
                    nc.sync.dma_start(out_r[img][:, :, sl], yout[:])
    return _split_waits(nc)


def _prep_weights(inputs):
    import ml_dtypes

    bf = ml_dtypes.bfloat16
    w_qkv = np.asarray(inputs["w_qkv"], np.float32)
    g1 = np.asarray(inputs["g1"], np.float32)
    g2 = np.asarray(inputs["g2"], np.float32)
    for name in ("beta1", "beta2", "b_qkv", "b_proj", "b_ffn1", "b_ffn2"):
        assert not np.any(np.asarray(inputs[name])), f"{name} nonzero unsupported"
    wg = w_qkv * g1[None, :]  # fold LN gamma into qkv weights
    wg3 = wg.reshape(NH, 3 * CH, C)
    wq = wg3[:, 0:CH, :]  # [NH, 48, C]
    wk = wg3[:, CH : 2 * CH, :]
    wv_ = wg3[:, 2 * CH : 3 * CH, :]
    # qk columns interleaved per head: j = h*96 + (0..47 q | 48..95 k)
    wqk = np.concatenate([wq, wk], axis=1).reshape(2 * C, C)  # [768, 384]
    wqk_t = np.ascontiguousarray(wqk.T).astype(bf)  # [384, 768]
    u_qk = wqk.sum(axis=1)[None, :].astype(bf)  # [1, 768]
    wv_t = np.ascontiguousarray(wv_.transpose(1, 0, 2)).astype(bf)
    # wpj48[d, h, o] = w_proj[o, 48h+d]
    wpj48 = np.ascontiguousarray(
        np.asarray(inputs["w_proj"], np.float32).T.reshape(NH, CH, C)
        .transpose(1, 0, 2)).astype(bf)
    w1g = np.asarray(inputs["w_ffn1"], np.float32) * g2[None, :]
    w1_t = np.ascontiguousarray(w1g.T).astype(bf)  # [384, 1536]
    w2_t = np.ascontiguousarray(
        np.asarray(inputs["w_ffn2"], np.float32).T).astype(bf)
    ls = np.asarray(inputs["logit_scale"], np.float32).reshape(NH)
    scale_row = np.exp(np.minimum(ls, LOGIT_MAX))[None, :]
    return dict(
        wqk_t=wqk_t, u_qk=np.ascontiguousarray(u_qk), wv=wv_t,
        wpj48=wpj48, w1_t=w1_t, w2_t=w2_t,
        scale_row=np.ascontiguousarray(scale_row))


def kernel(**inputs):
    from concourse.bass_utils import run_bass_kernel_spmd

    if "nc" not in _CACHE:
        _CACHE["nc"] = _build_nc()
    nc = _CACHE["nc"]

    x = np.asarray(inputs["x"], np.float32).reshape(B, C, N)
    wmap = _prep_weights(inputs)
    in_maps = []
    for c in range(NCORES):
        m = dict(wmap)
        m["xs"] = np.ascontiguousarray(x[c * BPC : (c + 1) * BPC])
        in_maps.append(m)
    res = run_bass_kernel_spmd(nc, in_maps, list(range(NCORES)))
    out = np.concatenate([r["out"] for r in res.results], axis=0)
    return out.reshape(B, C, 64, 64).astype(np.float32)
